# revision 11
# baseline (speedup 1.0000x reference)
"""ConvAttention Trainium2 kernel.

Reference computation (per batch b):
  q,k,v,p = conv(1x5,pad2)+BN(eval)+LeakyReLU(0.3) of x   (B,64,16,512)
  attn = softmax(einsum("chw,chv->cwv", q, k), axis=-1)
  out  = einsum("chw,cwv->chv", v, attn) + p

Sharding: data-parallel over batch B=16 across 8 cores (2 per core).

Device algorithm per core (all matmuls in float32r, N=512 -> 1 cyc/row):
  - BN folded into conv weights/shifts on host.
  - conv as matmul: stationary W [K=(dw,ci), M=(branch,co)], streaming
    x-patches.  dw=4 K-rows reuse the dw=3 rows of the patch buffer with a
    +1 element offset (guard columns make the shifted read safe).
  - LeakyReLU via 2 DVE ops (tensor_scalar + scalar_tensor_tensor).
  - q,k,p reshaped via SBUF->SBUF DMA to "Q-layout": per channel c
    (grouped 4/super-tile) rows (32*(c%4)+h), free w.
  - v transposed via PE into V-layout [w, (c,h)].
  - S = q^T k per channel: K=16 row-tiled matmuls (tile_position=(32j,0)).
  - E = exp(S) on ACT with accum_out giving the softmax row-sums free.
  - softmax normalization folded into v columns (rinv = 1/rowsum).
  - out = v' @ E: col-tiled matmuls (tile_position=(0,32j)), PSUM-accumulated
    over 4 w-chunks, + pe, DMA out.
"""

import numpy as np

import concourse.bass as bass
from concourse import bacc
import concourse.mybir as mybir
import concourse.tile as tile
from concourse.bass_utils import run_bass_kernel_spmd
from concourse.masks import make_identity

F32 = mybir.dt.float32
F32R = mybir.dt.float32r
BF16 = mybir.dt.bfloat16

B, CIN, COUT, H, W = 16, 32, 64, 16, 512
NCORES = 8
BPC = B // NCORES          # batches per core = 2
KW = 5
LRELU = 0.3
BN_EPS = 1e-5
WPITCH = 520               # xpatch row pitch: 512 + 8 guard cols
NCG = COUT // 4            # 16 channel groups of 4

TRACE = False              # set by test.py for profiling runs
_TRACE_KW = {}


def r32(ap):
    return ap.bitcast(F32R)


def build_nc():
    nc = bacc.Bacc(None)

    xs = nc.dram_tensor("xs", [BPC, 128, H * WPITCH], F32R, kind="ExternalInput")
    wa = nc.dram_tensor("wa", [128, 256], F32R, kind="ExternalInput")
    wb = nc.dram_tensor("wb", [128, 256], F32R, kind="ExternalInput")
    sh = nc.dram_tensor("sh", [128, 2], F32, kind="ExternalInput")
    out = nc.dram_tensor("out", [BPC, COUT, H, W], F32, kind="ExternalOutput")

    with tile.TileContext(nc) as tc:
        with (
            tc.tile_pool(name="const", bufs=1) as constp,
            tc.tile_pool(name="xp", bufs=1) as xpp,
            tc.tile_pool(name="fa", bufs=1) as fap,
            tc.tile_pool(name="sup", bufs=1) as supp,
            tc.tile_pool(name="vsup", bufs=1) as vsupp,
            tc.tile_pool(name="t1", bufs=2) as t1p,
            tc.tile_pool(name="e", bufs=6) as ep,
            tc.tile_pool(name="vtmp", bufs=2) as vtp,
            tc.tile_pool(name="osb", bufs=2) as osbp,
            tc.tile_pool(name="rs", bufs=2) as rsp,
            tc.tile_pool(name="psA", bufs=6, space="PSUM") as psA,
            tc.tile_pool(name="psB", bufs=2, space="PSUM") as psB,
        ):
            # ---- constants ----
            w_a = constp.tile([128, 256], F32R, tag="wa")
            w_b = constp.tile([128, 256], F32R, tag="wb")
            shifts = constp.tile([128, 2], F32, tag="sh")
            ident_f = constp.tile([128, 128], F32, tag="idf")
            ident = constp.tile([128, 128], F32R, tag="id")
            nc.sync.dma_start(w_a[:, :], wa[:, :])
            nc.sync.dma_start(w_b[:, :], wb[:, :])
            nc.sync.dma_start(shifts[:, :], sh[:, :])
            make_identity(nc, ident_f[:, :])
            nc.vector.tensor_copy(ident[:, :], ident_f[:, :])

            # ---- persistent buffers (reused across the 2 batches) ----
            # x-patches (host-built): partition (dw*32+ci) for dw<4;
            # free h*WPITCH+w, zero borders + guard cols baked in.
            xpat = xpp.tile([128, H * WPITCH], F32R, tag="xpat")

            # conv output staging [co', (h,w)] (one M-chunk at a time)
            fa = fap.tile([128, H * W], F32R, tag="fa")

            # Q-layout supers: partition 32*(c%4)+h, free (c//4)*512 + w
            sup_q = supp.tile([128, NCG * W], F32R, tag="supq")
            sup_k = supp.tile([128, NCG * W], F32R, tag="supk")
            sup_p = supp.tile([128, NCG * W], F32R, tag="supp")
            nc.gpsimd.memset(sup_p[:, :].bitcast(mybir.dt.uint32), 0)  # pad rows

            # V-layout: per w-chunk [128 w, 64c*16h]
            vsup = [vsupp.tile([128, COUT * H], F32R, tag=f"vs{wc}",
                                name=f"vsup{wc}")
                    for wc in range(4)]
            vt = vsupp.tile([128, 16 * 32], BF16, tag="vt")
            nc.gpsimd.memset(vt[:, :], 0.0)

            for b in range(BPC):
                # ---------- x-patch load (host pre-patched) ----------
                nc.sync.dma_start(xpat[:, :], xs[b, :, :])

                # ---------- conv: 2 M-chunks of (branch,co) ----------
                # mch 0 -> q,k ; mch 1 -> v,p
                for mch in range(2):
                    for h in range(H):
                        cp = psA.tile([128, W], F32, tag="ps")
                        nc.tensor.matmul(
                            cp[:, :],
                            (w_a[:, mch * 128:(mch + 1) * 128]),
                            (xpat[:, h * WPITCH: h * WPITCH + W]),
                            start=True, stop=False)
                        # dw=4 rows: reuse dw=3 patch rows shifted by +1
                        nc.tensor.matmul(
                            cp[:, :],
                            (w_b[96:128, mch * 128:(mch + 1) * 128]),
                            (xpat[96:128, h * WPITCH + 1: h * WPITCH + 1 + W]),
                            start=False, stop=True,
                            tile_position=(96, 0))
                        # epilogue: y = max(cp+shift, 0.3*(cp+shift))
                        sh_ap = shifts[:, mch: mch + 1]
                        t1 = t1p.tile([128, W], F32, tag="t1")
                        nc.vector.tensor_scalar(
                            t1[:, :], cp[:, :], sh_ap, LRELU,
                            op0=mybir.AluOpType.add, op1=mybir.AluOpType.mult)
                        nc.vector.scalar_tensor_tensor(
                            fa[:, h * W:(h + 1) * W], cp[:, :], sh_ap, t1[:, :],
                            op0=mybir.AluOpType.add, op1=mybir.AluOpType.max)

                    if mch == 0:
                        # reshape q (fa rows 0:64), k (rows 64:128) -> supers
                        for br, sup in ((0, sup_q), (1, sup_k)):
                            for j in range(4):
                                for cg in range(NCG):
                                    c = cg * 4 + j
                                    nc.sync.dma_start(
                                        sup[32 * j:32 * j + 16,
                                            cg * W:(cg + 1) * W],
                                        fa[br * 64 + c: br * 64 + c + 1, :])
                    else:
                        # pe (rows 64:128) -> super
                        for j in range(4):
                            for cg in range(NCG):
                                c = cg * 4 + j
                                nc.sync.dma_start(
                                    sup_p[32 * j:32 * j + 16,
                                          cg * W:(cg + 1) * W],
                                    fa[64 + c: 64 + c + 1, :])
                        # v (rows 0:64): PE transpose per (h, w-chunk)
                        for h in range(H):
                            for wc in range(4):
                                tp = psA.tile([128, 64], F32R, tag="ps")
                                nc.tensor.transpose(
                                    tp[:, :],
                                    fa[0:64, h * W + wc * 128:
                                       h * W + wc * 128 + 128],
                                    ident[0:64, 0:64])
                                nc.vector.tensor_copy(
                                    vsup[wc][:, h::H], tp[:, :])

                # ---------- attention per channel group ----------
                for cg in range(NCG):
                    rs = rsp.tile([128, 16], F32, tag="rs")
                    rinv = rsp.tile([128, 16], F32, tag="rinv")
                    opsum = psB.tile([128, W], F32, tag="op")
                    osb = osbp.tile([128, W], F32, tag="osb")
                    for j in range(4):
                        c = cg * 4 + j
                        etiles = []
                        for wc in range(4):
                            sp = psA.tile([128, W], F32, tag="ps")
                            nc.tensor.matmul(
                                sp[:, :],
                                (sup_q[32 * j:32 * j + 16,
                                          cg * W + wc * 128:
                                          cg * W + wc * 128 + 128]),
                                (sup_k[32 * j:32 * j + 16,
                                          cg * W:(cg + 1) * W]),
                                start=True, stop=True,
                                tile_position=(32 * j, 0))
                            et = ep.tile([128, W], BF16, tag="e")
                            nc.scalar.activation(
                                et[:, :], sp[:, :],
                                mybir.ActivationFunctionType.Exp,
                                accum_out=rs[:, j * 4 + wc: j * 4 + wc + 1])
                            etiles.append(et)
                        nc.vector.reciprocal(rinv[:, j * 4: j * 4 + 4],
                                             rs[:, j * 4: j * 4 + 4])
                        for wc in range(4):
                            sl = (j * 4 + wc) * 32
                            nc.vector.tensor_scalar_mul(
                                vt[:, sl: sl + 16],
                                vsup[wc][:, c * 16:(c + 1) * 16],
                                rinv[:, j * 4 + wc: j * 4 + wc + 1])
                            nc.tensor.matmul(
                                opsum[32 * j: 32 * j + 32, :],
                                (vt[:, sl: sl + 32]),
                                etiles[wc][:, :],
                                start=(wc == 0), stop=(wc == 3),
                                tile_position=(0, 32 * j))
                    nc.vector.tensor_tensor(
                        osb[:, :], opsum[:, :],
                        sup_p[:, cg * W:(cg + 1) * W].bitcast(F32),
                        op=mybir.AluOpType.add)
                    for j in range(4):
                        nc.sync.dma_start(
                            out[b, cg * 4 + j, :, :],
                            osb[32 * j: 32 * j + 16, :])
    if not nc.is_finalized():
        nc.finalize()
    return nc


def _host_weights(inputs):
    wa = np.zeros((128, 256), np.float32)
    wb = np.zeros((128, 256), np.float32)
    sh = np.zeros((128, 2), np.float32)
    for bi, br in enumerate(["q", "k", "v", "p"]):
        w = np.asarray(inputs[f"w_{br}"], np.float32)       # (64,32,1,5)
        gamma = np.asarray(inputs[f"gamma_{br}"], np.float32)
        beta = np.asarray(inputs[f"beta_{br}"], np.float32)
        mean = np.asarray(inputs[f"mean_{br}"], np.float32)
        var = np.asarray(inputs[f"var_{br}"], np.float32)
        scale = gamma / np.sqrt(var + BN_EPS)
        shift = beta - mean * scale
        wsc = w[:, :, 0, :] * scale[:, None, None]          # (64,32,5)
        mch, coff = bi // 2, (bi % 2) * 64
        for dw in range(4):
            # wa[dw*32+ci, mch*128+coff+co] = wsc[co, ci, dw]
            wa[dw * 32:(dw + 1) * 32, mch * 128 + coff: mch * 128 + coff + 64] = \
                wsc[:, :, dw].T
        wb[96:128, mch * 128 + coff: mch * 128 + coff + 64] = wsc[:, :, 4].T
        sh[coff: coff + 64, mch] = shift
    return wa, wb, sh


_NC_CACHE = {}


def _host_xpatch(x):
    # x: (B, 32, H, W) -> (B, 128, H*WPITCH): row dw*32+ci holds
    # x[:, ci, h, w+dw-2] (zeros outside), 8 zero guard cols per h row.
    xp = np.zeros((B, 128, H, WPITCH), np.float32)
    for dw in range(4):
        lo = max(0, 2 - dw)
        hi = W - max(0, dw - 2)
        xp[:, dw * 32:(dw + 1) * 32, :, lo:hi] = \
            x[:, :, :, lo + dw - 2: hi + dw - 2]
    return xp.reshape(B, 128, H * WPITCH)


def kernel(**inputs):
    x = np.ascontiguousarray(np.asarray(inputs["x"], np.float32))
    xp = _host_xpatch(x)
    wa, wb, sh = _host_weights(inputs)
    if "nc" not in _NC_CACHE:
        _NC_CACHE["nc"] = build_nc()
    nc = _NC_CACHE["nc"]
    in_maps = [
        {"xs": np.ascontiguousarray(xp[i * BPC:(i + 1) * BPC]),
         "wa": wa, "wb": wb, "sh": sh}
        for i in range(NCORES)
    ]
    res = run_bass_kernel_spmd(nc, in_maps, list(range(NCORES)),
                               trace=TRACE, **_TRACE_KW)
    if TRACE:
        print(f"HW exec time: {res.exec_time_ns} ns")
        print(f"mean exec time: {res.mean_exec_time_ns} ns")
        if res.instructions_and_trace is not None:
            print(f"trace: {res.instructions_and_trace[1]}")
    return np.concatenate([r["out"] for r in res.results], axis=0)
